# revision 10
# baseline (speedup 1.0000x reference)
"""MultiHeadCrossAttention Trainium2 kernel (8 NeuronCores, SPMD).

Sharding: core c = (batch b=c//4, head-group hg=c%4) -- 4 heads of d=64 each.
Per core: qT/kT/v projections (weights pre-transposed + mean-centered on host so
LayerNorm mean-subtraction is free), LN variance via PE ones-block reduce,
attention with S^T layout ([keys, q], softmax denominator via a ones column
appended to v in the AV matmul), and the head-group partial of the output
projection. Host sums the 4 partials per batch and adds the bias.
"""

import os
import sys

sys.path.insert(0, "/opt/trn_rl_repo")

import numpy as np
import ml_dtypes

N_HEADS = 16
D = 64            # head dim
EMB = 1024
CTX = 1024
B = 2
SQ = 2048
SK = 2048
HG = 4            # heads per core
INNER_C = HG * D  # 256 inner dims per core
EPS = 1e-5
SCALE = 1.0 / 8.0  # 1/sqrt(64)
P = 128

_cached_nc = None


def _build():
    import concourse.bass as bass  # noqa: F401
    import concourse.tile as tile
    from concourse import mybir, bacc
    from contextlib import ExitStack

    f32 = mybir.dt.float32
    bf16 = mybir.dt.bfloat16
    AF = mybir.ActivationFunctionType
    OP = mybir.AluOpType

    nc = bacc.Bacc(None, target_bir_lowering=False, debug=False, num_devices=8)

    embT_d = nc.dram_tensor("embT", [EMB, SQ], f32, kind="ExternalInput")
    ctxT_d = nc.dram_tensor("ctxT", [CTX, SK], f32, kind="ExternalInput")
    wqT_d = nc.dram_tensor("wqT", [EMB, INNER_C], f32, kind="ExternalInput")
    wkT_d = nc.dram_tensor("wkT", [CTX, INNER_C], f32, kind="ExternalInput")
    wvT_d = nc.dram_tensor("wvT", [CTX, INNER_C], f32, kind="ExternalInput")
    wuT_d = nc.dram_tensor("wuT", [INNER_C, EMB], f32, kind="ExternalInput")
    red_d = nc.dram_tensor("redblk", [P, 2], bf16, kind="ExternalInput")
    qnw_d = nc.dram_tensor("qnw", [P, 1], f32, kind="ExternalInput")
    qnb_d = nc.dram_tensor("qnb", [P, 1], f32, kind="ExternalInput")
    knw_d = nc.dram_tensor("knw", [P, 1], f32, kind="ExternalInput")
    knb_d = nc.dram_tensor("knb", [P, 1], f32, kind="ExternalInput")
    y_d = nc.dram_tensor("ypart", [SQ, EMB], f32, kind="ExternalOutput")
    dbg = os.environ.get("KERNEL_DEBUG")
    if dbg:
        qTn_d = nc.dram_tensor("dbg_qTn", [P, 2, SQ], f32, kind="ExternalOutput")
        kTn_d = nc.dram_tensor("dbg_kTn", [P, 2, SK], f32, kind="ExternalOutput")
        v_dd = nc.dram_tensor("dbg_v", [P, 16, HG * 65], f32, kind="ExternalOutput")
        oT_d = nc.dram_tensor("dbg_oT", [D, HG, SQ], f32, kind="ExternalOutput")

    with tile.TileContext(nc) as tc, ExitStack() as top:
        consts = top.enter_context(tc.tile_pool(name="consts", bufs=1))
        red_sb = consts.tile([P, 2], bf16)
        nc.sync.dma_start(red_sb[:], red_d[:])
        qnw_sb = consts.tile([P, 1], f32)
        nc.sync.dma_start(qnw_sb[:], qnw_d[:])
        qnb_sb = consts.tile([P, 1], f32)
        nc.sync.dma_start(qnb_sb[:], qnb_d[:])
        knw_sb = consts.tile([P, 1], f32)
        nc.sync.dma_start(knw_sb[:], knw_d[:])
        knb_sb = consts.tile([P, 1], f32)
        nc.sync.dma_start(knb_sb[:], knb_d[:])
        eps_sb = consts.tile([2, 1], f32)
        nc.vector.memset(eps_sb[:], EPS)

        # persistent SBUF tensors
        persist = top.enter_context(tc.tile_pool(name="persist", bufs=1))
        qTn_sb = persist.tile([P, 2, SQ], f32)     # [p, mc, q] normalized q^T
        kTn_sb = persist.tile([P, 2, SK], f32)
        v_sb = persist.tile([P, 16, HG * 65], bf16)  # per sk-tile: 4x[v_h|1]
        oT_sb = persist.tile([D, HG, SQ], f32)     # unnorm-then-normalized O^T
        wuT_sb = persist.tile([D, HG, EMB], f32)   # per-head Wu cols^T
        nc.sync.dma_start(
            wuT_sb[:], wuT_d[:].rearrange("(h p) e -> p h e", p=D)
        )
        # ones columns of v
        nc.vector.memset(
            v_sb.rearrange("p k (g c) -> p k g c", c=65)[:, :, :, 64:65], 1.0
        )

        # ---------------- Stage A: projections + layernorm ----------------
        def project_norm(xT_sb, wT_sb, out_sb, w_ap, b_ap, proj_ps, var_pool,
                         sq_pool, small, bc_pool, dram_bnc):
            for mc in range(2):
                var_ps = var_pool.tile([2, SQ], f32)
                for n in range(4):
                    pp = proj_ps.tile([P, 512], f32)
                    for k in range(8):
                        nc.tensor.matmul(
                            pp[:],
                            wT_sb[:, k, 128 * mc:128 * mc + 128],
                            xT_sb[:, k, 512 * n:512 * n + 512],
                            start=(k == 0),
                            stop=(k == 7),
                        )
                    sq = sq_pool.tile([P, 512], bf16)
                    nc.scalar.activation(sq[:], pp[:], AF.Square)
                    nc.tensor.matmul(
                        var_ps[:, 512 * n:512 * n + 512], red_sb[:], sq[:],
                        start=True, stop=True,
                    )
                    nc.vector.tensor_copy(
                        out_sb[:, mc, 512 * n:512 * n + 512], pp[:]
                    )
                srt = small.tile([2, SQ], f32)
                nc.scalar.activation(srt[:], var_ps[:], AF.Sqrt, bias=eps_sb[:])
                rs = small.tile([2, SQ], f32, tag="rs")
                nc.vector.reciprocal_approx_fast(rs[:], srt[:])
                rsd = dram_bnc.tile([2, SQ], f32)
                nc.sync.dma_start(rsd[:], rs[:])
                rsb = bc_pool.tile([P, SQ], f32)
                nc.sync.dma_start(rsb[0:64, :], rsd[0:1, :].to_broadcast((64, SQ)))
                nc.sync.dma_start(rsb[64:128, :], rsd[1:2, :].to_broadcast((64, SQ)))
                nc.vector.scalar_tensor_tensor(
                    out_sb[:, mc, :], out_sb[:, mc, :], w_ap, rsb[:],
                    op0=OP.mult, op1=OP.mult,
                )
                nc.vector.tensor_scalar_add(out_sb[:, mc, :], out_sb[:, mc, :], b_ap)

        with ExitStack() as sa:
            proj_ps = sa.enter_context(
                tc.tile_pool(name="proj_ps", bufs=2, space="PSUM"))
            var_pool = sa.enter_context(
                tc.tile_pool(name="var_ps", bufs=1, space="PSUM"))
            sq_pool = sa.enter_context(tc.tile_pool(name="sq", bufs=3))
            small = sa.enter_context(tc.tile_pool(name="small", bufs=1))
            bc_pool = sa.enter_context(tc.tile_pool(name="bc", bufs=1))
            dram_bnc = sa.enter_context(
                tc.tile_pool(name="dram_bnc", bufs=2, space="DRAM"))

            with ExitStack() as sa1:
                embw = sa1.enter_context(tc.tile_pool(name="embw", bufs=1))
                embT_sb = embw.tile([P, 8, SQ], f32)
                for k in range(8):
                    nc.sync.dma_start(
                        embT_sb[:, k, :],
                        embT_d[:].rearrange("(k p) q -> p k q", p=P)[:, k, :],
                    )
                wq_sb = embw.tile([P, 8, INNER_C], f32, tag="wq")
                nc.sync.dma_start(
                    wq_sb[:], wqT_d[:].rearrange("(k p) m -> p k m", p=P)
                )
                project_norm(embT_sb, wq_sb, qTn_sb, qnw_sb[:], qnb_sb[:],
                             proj_ps, var_pool, sq_pool, small, bc_pool,
                             dram_bnc)

            with ExitStack() as sa2:
                ctxw = sa2.enter_context(tc.tile_pool(name="ctxw", bufs=1))
                ctxT_sb = ctxw.tile([P, 8, SK], f32)
                for k in range(8):
                    nc.sync.dma_start(
                        ctxT_sb[:, k, :],
                        ctxT_d[:].rearrange("(k p) q -> p k q", p=P)[:, k, :],
                    )
                wk_sb = ctxw.tile([P, 8, INNER_C], f32, tag="wk")
                nc.sync.dma_start(
                    wk_sb[:], wkT_d[:].rearrange("(k p) m -> p k m", p=P)
                )
                wv_sb = ctxw.tile([P, 8, INNER_C], f32, tag="wv")
                nc.sync.dma_start(
                    wv_sb[:], wvT_d[:].rearrange("(k p) m -> p k m", p=P)
                )
                project_norm(ctxT_sb, wk_sb, kTn_sb, knw_sb[:], knb_sb[:],
                             proj_ps, var_pool, sq_pool, small, bc_pool,
                             dram_bnc)

                # v projection: v[sk, m] natural layout, + ones columns
                with tc.tile_pool(name="vproj_ps", bufs=2, space="PSUM") as vps:
                    for sk in range(16):
                        vp = vps.tile([P, INNER_C], f32)
                        for k in range(8):
                            nc.tensor.matmul(
                                vp[:],
                                ctxT_sb[:, k, 128 * sk:128 * sk + 128],
                                wv_sb[:, k, :],
                                start=(k == 0),
                                stop=(k == 7),
                            )
                        nc.vector.tensor_copy(
                            v_sb.rearrange("p k (g c) -> p k g c", c=65)
                            [:, sk, :, 0:64],
                            vp[:].rearrange("p (g c) -> p g c", c=64),
                        )

        # ---------------- Stage B: attention + output projection ----------
        with ExitStack() as sb:
            st_ps = sb.enter_context(
                tc.tile_pool(name="st_ps", bufs=2, space="PSUM"))
            ot_ps = sb.enter_context(
                tc.tile_pool(name="ot_ps", bufs=2, space="PSUM"))
            y_ps = sb.enter_context(
                tc.tile_pool(name="y_ps", bufs=1, space="PSUM"))
            at_pool = sb.enter_context(tc.tile_pool(name="at", bufs=18))
            dr_pool = sb.enter_context(tc.tile_pool(name="dr", bufs=4))
            obc_pool = sb.enter_context(tc.tile_pool(name="obc", bufs=4))
            dramb = sb.enter_context(
                tc.tile_pool(name="dramb", bufs=4, space="DRAM"))
            yout = sb.enter_context(tc.tile_pool(name="yout", bufs=3))

            for qh in range(2):
                for hp in range(2):
                    for h2 in range(2):
                        h = 2 * hp + h2
                        po = 64 * h2
                        at_tiles = []
                        for kt in range(16):
                            sp = st_ps.tile([P, 1024], f32)
                            for qn in range(2):
                                nc.tensor.matmul(
                                    sp[:, 512 * qn:512 * qn + 512],
                                    kTn_sb[po:po + 64, hp,
                                           128 * kt:128 * kt + 128],
                                    qTn_sb[po:po + 64, hp,
                                           1024 * qh + 512 * qn:
                                           1024 * qh + 512 * qn + 512],
                                    start=True, stop=True,
                                )
                            at = at_pool.tile([P, 1024], bf16)
                            nc.scalar.activation(at[:], sp[:], AF.Exp,
                                                 scale=SCALE)
                            at_tiles.append(at)
                        for qc2 in range(2):
                            qc = 2 * qh + qc2
                            ot = ot_ps.tile([65, 512], f32)
                            for kt in range(16):
                                nc.tensor.matmul(
                                    ot[:],
                                    v_sb[:, kt, 65 * h:65 * h + 65],
                                    at_tiles[kt][:, 512 * qc2:512 * qc2 + 512],
                                    start=(kt == 0),
                                    stop=(kt == 15),
                                )
                            dr = dr_pool.tile([65, 512], f32)
                            dr2 = dr_pool.tile([65, 512], f32, tag="dr2")
                            nc.vector.tensor_copy(dr[64:65, :], ot[64:65, :])
                            nc.vector.reciprocal(dr2[64:65, :], dr[64:65, :])
                            drd = dramb.tile([1, 512], f32)
                            nc.sync.dma_start(drd[:], dr2[64:65, :])
                            obc = obc_pool.tile([64, 512], f32)
                            nc.sync.dma_start(
                                obc[:], drd[0:1, :].to_broadcast((64, 512)))
                            nc.vector.tensor_mul(
                                oT_sb[:, h, 512 * qc:512 * qc + 512],
                                ot[0:64, :], obc[:],
                            )
                # output projection for the two completed q-chunks
                for qc2 in range(2):
                    qc = 2 * qh + qc2
                    for qm in range(4):
                        q0 = 512 * qc + 128 * qm
                        yp = y_ps.tile([P, 1024], f32)
                        for h in range(4):
                            for n2 in range(2):
                                nc.tensor.matmul(
                                    yp[:, 512 * n2:512 * n2 + 512],
                                    oT_sb[:, h, q0:q0 + 128],
                                    wuT_sb[:, h, 512 * n2:512 * n2 + 512],
                                    start=(h == 0),
                                    stop=(h == 3),
                                )
                        ysb = yout.tile([P, 1024], f32)
                        nc.vector.tensor_copy(ysb[:], yp[:])
                        nc.sync.dma_start(y_d[q0:q0 + 128, :], ysb[:])

        if dbg:
            nc.sync.dma_start(qTn_d[:], qTn_sb[:])
            nc.sync.dma_start(kTn_d[:], kTn_sb[:])
            with tc.tile_pool(name="vdbg", bufs=1) as vdbg:
                vf = vdbg.tile([P, 16, HG * 65], f32)
                nc.vector.tensor_copy(vf[:], v_sb[:])
                nc.sync.dma_start(v_dd[:], vf[:])
            nc.sync.dma_start(oT_d[:], oT_sb[:])

    nc.compile()
    return nc


def _host_inputs(emb, context, Wq, Wk, Wv, Wu, qn_w, qn_b, kn_w, kn_b):
    bf16 = ml_dtypes.bfloat16
    redblk = np.zeros((P, 2), np.float32)
    redblk[0:64, 0] = 1.0 / 64.0
    redblk[64:128, 1] = 1.0 / 64.0
    redblk = redblk.astype(bf16)

    def center(Wrows):
        Wh = Wrows.reshape(HG, D, Wrows.shape[1])
        return (Wh - Wh.mean(axis=1, keepdims=True)).reshape(Wrows.shape)

    f32c = lambda a: np.ascontiguousarray(a, dtype=np.float32)
    tile2 = lambda w: np.ascontiguousarray(
        np.tile(np.asarray(w, np.float32), 2)[:, None])

    in_maps = []
    for c in range(8):
        b, hg = divmod(c, 4)
        rows = slice(INNER_C * hg, INNER_C * (hg + 1))
        in_maps.append({
            "embT": f32c(emb[b].T),
            "ctxT": f32c(context[b].T),
            "wqT": f32c(center(Wq[rows]).T),
            "wkT": f32c(center(Wk[rows]).T),
            "wvT": f32c(Wv[rows].T),
            "wuT": f32c(Wu[:, rows].T),
            "redblk": redblk,
            "qnw": tile2(qn_w),
            "qnb": tile2(qn_b),
            "knw": tile2(kn_w),
            "knb": tile2(kn_b),
        })
    return in_maps


def kernel(emb, context, Wq, Wk, Wv, Wu, bu, qn_w, qn_b, kn_w, kn_b):
    from concourse.bass_utils import run_bass_kernel_spmd

    global _cached_nc
    if _cached_nc is None:
        _cached_nc = _build()
    nc = _cached_nc

    emb = np.asarray(emb, np.float32)
    context = np.asarray(context, np.float32)
    in_maps = _host_inputs(np.asarray(emb), np.asarray(context),
                           np.asarray(Wq), np.asarray(Wk), np.asarray(Wv),
                           np.asarray(Wu), np.asarray(qn_w), np.asarray(qn_b),
                           np.asarray(kn_w), np.asarray(kn_b))

    trace = bool(os.environ.get("KERNEL_TRACE"))
    res = run_bass_kernel_spmd(nc, in_maps, core_ids=list(range(8)),
                               trace=trace)
    if trace:
        print(f"HW exec time: {res.exec_time_ns} ns")

    out = np.zeros((B, SQ, EMB), np.float32)
    for c in range(8):
        out[c // 4] += res.results[c]["ypart"]
    out += np.asarray(bu, np.float32)[None, None, :]
    return out


if __name__ == "__main__":
    rng = np.random.default_rng(0)
    pass


# revision 11
# speedup vs baseline: 1.4514x; 1.4514x over previous
"""MultiHeadCrossAttention Trainium2 kernel (8 NeuronCores, SPMD).

Sharding: core c = (batch b=c//4, head-group hg=c%4) -- 4 heads of d=64 each.
Per core: qT/kT/v projections (weights pre-transposed + mean-centered on host so
LayerNorm mean-subtraction is free), LN variance via PE ones-block reduce,
attention with S^T layout ([keys, q], softmax denominator via a ones column
appended to v in the AV matmul), and the head-group partial of the output
projection. Host sums the 4 partials per batch and adds the bias.
"""

import os
import sys

sys.path.insert(0, "/opt/trn_rl_repo")

import numpy as np
import ml_dtypes

N_HEADS = 16
D = 64            # head dim
EMB = 1024
CTX = 1024
B = 2
SQ = 2048
SK = 2048
HG = 4            # heads per core
INNER_C = HG * D  # 256 inner dims per core
EPS = 1e-5
SCALE = 1.0 / 8.0  # 1/sqrt(64)
P = 128

_cached_nc = None


def _build():
    import concourse.bass as bass  # noqa: F401
    import concourse.tile as tile
    from concourse import mybir, bacc
    from contextlib import ExitStack

    f32 = mybir.dt.float32
    bf16 = mybir.dt.bfloat16
    AF = mybir.ActivationFunctionType
    OP = mybir.AluOpType

    nc = bacc.Bacc(None, target_bir_lowering=False, debug=False, num_devices=8)

    embT_d = nc.dram_tensor("embT", [EMB, SQ], f32, kind="ExternalInput")
    ctxT_d = nc.dram_tensor("ctxT", [CTX, SK], f32, kind="ExternalInput")
    wqT_d = nc.dram_tensor("wqT", [EMB, INNER_C], f32, kind="ExternalInput")
    wkT_d = nc.dram_tensor("wkT", [CTX, INNER_C], f32, kind="ExternalInput")
    wvT_d = nc.dram_tensor("wvT", [CTX, INNER_C], f32, kind="ExternalInput")
    wuT_d = nc.dram_tensor("wuT", [INNER_C, EMB], f32, kind="ExternalInput")
    red_d = nc.dram_tensor("redblk", [P, 2], bf16, kind="ExternalInput")
    qnw_d = nc.dram_tensor("qnw", [P, 1], f32, kind="ExternalInput")
    qnb_d = nc.dram_tensor("qnb", [P, 1], f32, kind="ExternalInput")
    knw_d = nc.dram_tensor("knw", [P, 1], f32, kind="ExternalInput")
    knb_d = nc.dram_tensor("knb", [P, 1], f32, kind="ExternalInput")
    y_d = nc.dram_tensor("ypart", [SQ, EMB], f32, kind="ExternalOutput")
    dbg = os.environ.get("KERNEL_DEBUG")
    if dbg:
        qTn_d = nc.dram_tensor("dbg_qTn", [P, 2, SQ], f32, kind="ExternalOutput")
        kTn_d = nc.dram_tensor("dbg_kTn", [P, 2, SK], f32, kind="ExternalOutput")
        v_dd = nc.dram_tensor("dbg_v", [P, 16, HG * 65], f32, kind="ExternalOutput")
        oT_d = nc.dram_tensor("dbg_oT", [D, HG, SQ], f32, kind="ExternalOutput")

    with tile.TileContext(nc) as tc, ExitStack() as top:
        consts = top.enter_context(tc.tile_pool(name="consts", bufs=1))
        red_sb = consts.tile([P, 2], bf16)
        nc.sync.dma_start(red_sb[:], red_d[:])
        qnw_sb = consts.tile([P, 1], f32)
        nc.sync.dma_start(qnw_sb[:], qnw_d[:])
        qnb_sb = consts.tile([P, 1], f32)
        nc.sync.dma_start(qnb_sb[:], qnb_d[:])
        knw_sb = consts.tile([P, 1], f32)
        nc.sync.dma_start(knw_sb[:], knw_d[:])
        knb_sb = consts.tile([P, 1], f32)
        nc.sync.dma_start(knb_sb[:], knb_d[:])
        eps_sb = consts.tile([2, 1], f32)
        nc.vector.memset(eps_sb[:], EPS)

        # persistent SBUF tensors
        persist = top.enter_context(tc.tile_pool(name="persist", bufs=1))
        qTn_sb = persist.tile([P, 2, SQ], f32)     # [p, mc, q] normalized q^T
        kTn_sb = persist.tile([P, 2, SK], f32)
        v_sb = persist.tile([P, 16, HG * 65], bf16)  # per sk-tile: 4x[v_h|1]
        oT_sb = persist.tile([D, HG, SQ], f32)     # unnorm-then-normalized O^T
        wuT_sb = persist.tile([D, HG, EMB], f32)   # per-head Wu cols^T
        nc.sync.dma_start(
            wuT_sb[:], wuT_d[:].rearrange("(h p) e -> p h e", p=D)
        )
        # ones columns of v
        nc.vector.memset(
            v_sb.rearrange("p k (g c) -> p k g c", c=65)[:, :, :, 64:65], 1.0
        )

        # ---------------- Stage A: projections + layernorm ----------------
        def project_norm(xT_sb, wT_sb, out_sb, w_ap, b_ap, proj_ps, var_pool,
                         sq_pool, small, bc_pool, dram_bnc):
            for mc in range(2):
                var_ps = var_pool.tile([2, SQ], f32)
                for n in range(4):
                    pp = proj_ps.tile([P, 512], f32)
                    for k in range(8):
                        nc.tensor.matmul(
                            pp[:],
                            wT_sb[:, k, 128 * mc:128 * mc + 128],
                            xT_sb[:, k, 512 * n:512 * n + 512],
                            start=(k == 0),
                            stop=(k == 7),
                        )
                    sq = sq_pool.tile([P, 512], bf16)
                    nc.scalar.activation(sq[:], pp[:], AF.Square)
                    nc.tensor.matmul(
                        var_ps[:, 512 * n:512 * n + 512], red_sb[:], sq[:],
                        start=True, stop=True,
                    )
                    nc.vector.tensor_copy(
                        out_sb[:, mc, 512 * n:512 * n + 512], pp[:]
                    )
                srt = small.tile([2, SQ], f32)
                nc.scalar.activation(srt[:], var_ps[:], AF.Sqrt, bias=eps_sb[:])
                rs = small.tile([2, SQ], f32, tag="rs")
                nc.vector.reciprocal_approx_fast(rs[:], srt[:])
                rsd = dram_bnc.tile([2, SQ], f32)
                nc.sync.dma_start(rsd[:], rs[:])
                rsb = bc_pool.tile([P, SQ], f32)
                nc.sync.dma_start(rsb[0:64, :], rsd[0:1, :].to_broadcast((64, SQ)))
                nc.sync.dma_start(rsb[64:128, :], rsd[1:2, :].to_broadcast((64, SQ)))
                nc.vector.scalar_tensor_tensor(
                    out_sb[:, mc, :], out_sb[:, mc, :], w_ap, rsb[:],
                    op0=OP.mult, op1=OP.mult,
                )
                nc.vector.tensor_scalar_add(out_sb[:, mc, :], out_sb[:, mc, :], b_ap)

        with ExitStack() as sa:
            proj_ps = sa.enter_context(
                tc.tile_pool(name="proj_ps", bufs=2, space="PSUM"))
            var_pool = sa.enter_context(
                tc.tile_pool(name="var_ps", bufs=1, space="PSUM"))
            sq_pool = sa.enter_context(tc.tile_pool(name="sq", bufs=3))
            small = sa.enter_context(tc.tile_pool(name="small", bufs=1))
            bc_pool = sa.enter_context(tc.tile_pool(name="bc", bufs=1))
            dram_bnc = sa.enter_context(
                tc.tile_pool(name="dram_bnc", bufs=2, space="DRAM"))

            with ExitStack() as sa1:
                embw = sa1.enter_context(tc.tile_pool(name="embw", bufs=1))
                embT_sb = embw.tile([P, 8, SQ], f32)
                for k in range(8):
                    nc.sync.dma_start(
                        embT_sb[:, k, :],
                        embT_d[:].rearrange("(k p) q -> p k q", p=P)[:, k, :],
                    )
                wq_sb = embw.tile([P, 8, INNER_C], f32, tag="wq")
                nc.sync.dma_start(
                    wq_sb[:], wqT_d[:].rearrange("(k p) m -> p k m", p=P)
                )
                project_norm(embT_sb, wq_sb, qTn_sb, qnw_sb[:], qnb_sb[:],
                             proj_ps, var_pool, sq_pool, small, bc_pool,
                             dram_bnc)

            with ExitStack() as sa2:
                ctxw = sa2.enter_context(tc.tile_pool(name="ctxw", bufs=1))
                ctxT_sb = ctxw.tile([P, 8, SK], f32)
                for k in range(8):
                    nc.sync.dma_start(
                        ctxT_sb[:, k, :],
                        ctxT_d[:].rearrange("(k p) q -> p k q", p=P)[:, k, :],
                    )
                wk_sb = ctxw.tile([P, 8, INNER_C], f32, tag="wk")
                nc.sync.dma_start(
                    wk_sb[:], wkT_d[:].rearrange("(k p) m -> p k m", p=P)
                )
                wv_sb = ctxw.tile([P, 8, INNER_C], f32, tag="wv")
                nc.sync.dma_start(
                    wv_sb[:], wvT_d[:].rearrange("(k p) m -> p k m", p=P)
                )
                project_norm(ctxT_sb, wk_sb, kTn_sb, knw_sb[:], knb_sb[:],
                             proj_ps, var_pool, sq_pool, small, bc_pool,
                             dram_bnc)

                # v projection: v[sk, m] natural layout, + ones columns
                with tc.tile_pool(name="vproj_ps", bufs=2, space="PSUM") as vps:
                    for sk in range(16):
                        vp = vps.tile([P, INNER_C], f32)
                        for k in range(8):
                            nc.tensor.matmul(
                                vp[:],
                                ctxT_sb[:, k, 128 * sk:128 * sk + 128],
                                wv_sb[:, k, :],
                                start=(k == 0),
                                stop=(k == 7),
                            )
                        nc.vector.tensor_copy(
                            v_sb.rearrange("p k (g c) -> p k g c", c=65)
                            [:, sk, :, 0:64],
                            vp[:].rearrange("p (g c) -> p g c", c=64),
                        )

        # ---------------- Stage B: attention + output projection ----------
        with ExitStack() as sb:
            st_ps = sb.enter_context(
                tc.tile_pool(name="st_ps", bufs=3, space="PSUM"))
            ot_ps = sb.enter_context(
                tc.tile_pool(name="ot_ps", bufs=2, space="PSUM"))
            at_pool = sb.enter_context(tc.tile_pool(name="at", bufs=36))
            den_pool = sb.enter_context(tc.tile_pool(name="den", bufs=2))
            obc_pool = sb.enter_context(tc.tile_pool(name="obc", bufs=4))
            dramb = sb.enter_context(
                tc.tile_pool(name="dramb", bufs=4, space="DRAM"))
            yout = sb.enter_context(tc.tile_pool(name="yout", bufs=3))

            for qh in range(2):
                for hp in range(2):
                    at_tiles = [[None] * 16, [None] * 16]
                    denall = den_pool.tile([65, 2048], f32)
                    for kt in range(16):
                        for h2 in range(2):
                            po = 64 * h2
                            sp = st_ps.tile([P, 1024], f32, tag="st")
                            for qn in range(2):
                                nc.tensor.matmul(
                                    sp[:, 512 * qn:512 * qn + 512],
                                    kTn_sb[po:po + 64, hp,
                                           128 * kt:128 * kt + 128],
                                    qTn_sb[po:po + 64, hp,
                                           1024 * qh + 512 * qn:
                                           1024 * qh + 512 * qn + 512],
                                    start=True, stop=True,
                                    tile_position=(po, 0),
                                )
                            at = at_pool.tile([P, 1024], bf16)
                            nc.scalar.activation(at[:], sp[:], AF.Exp,
                                                 scale=SCALE)
                            at_tiles[h2][kt] = at
                    for h2 in range(2):
                        h = 2 * hp + h2
                        for qc2 in range(2):
                            qc = 2 * qh + qc2
                            ot = ot_ps.tile([65, 512], f32)
                            for kt in range(16):
                                nc.tensor.matmul(
                                    ot[:],
                                    v_sb[:, kt, 65 * h:65 * h + 65],
                                    at_tiles[h2][kt][:, 512 * qc2:512 * qc2 + 512],
                                    start=(kt == 0),
                                    stop=(kt == 15),
                                )
                            j = 2 * h2 + qc2
                            nc.vector.tensor_copy(
                                denall[64:65, 512 * j:512 * j + 512],
                                ot[64:65, :])
                            nc.vector.tensor_copy(
                                oT_sb[:, h, 512 * qc:512 * qc + 512],
                                ot[0:64, :])
                    # batched reciprocal of the 4 denominator rows
                    dend = dramb.tile([1, 2048], f32)
                    nc.sync.dma_start(dend[:], denall[64:65, :])
                    den0 = den_pool.tile([4, 512], f32, tag="den0")
                    nc.sync.dma_start(
                        den0[:],
                        dend[0:1, :].rearrange("p (i c) -> (p i) c", c=512))
                    den0r = den_pool.tile([4, 512], f32, tag="den0r")
                    nc.vector.reciprocal_approx_fast(den0r[:], den0[:])
                    dend2 = dramb.tile([4, 512], f32, tag="dend2")
                    nc.sync.dma_start(dend2[:], den0r[:])
                    for h2 in range(2):
                        h = 2 * hp + h2
                        for qc2 in range(2):
                            qc = 2 * qh + qc2
                            j = 2 * h2 + qc2
                            obc = obc_pool.tile([64, 512], f32)
                            nc.sync.dma_start(
                                obc[:],
                                dend2[j:j + 1, :].to_broadcast((64, 512)))
                            nc.vector.tensor_mul(
                                oT_sb[:, h, 512 * qc:512 * qc + 512],
                                oT_sb[:, h, 512 * qc:512 * qc + 512],
                                obc[:])
                # output projection for the two completed q-chunks
                for qc2 in range(2):
                    qc = 2 * qh + qc2
                    for qm in range(4):
                        q0 = 512 * qc + 128 * qm
                        yp = st_ps.tile([P, 1024], f32, tag="st")
                        for h in range(4):
                            for n2 in range(2):
                                nc.tensor.matmul(
                                    yp[:, 512 * n2:512 * n2 + 512],
                                    oT_sb[:, h, q0:q0 + 128],
                                    wuT_sb[:, h, 512 * n2:512 * n2 + 512],
                                    start=(h == 0),
                                    stop=(h == 3),
                                )
                        ysb = yout.tile([P, 1024], f32)
                        nc.vector.tensor_copy(ysb[:], yp[:])
                        nc.sync.dma_start(y_d[q0:q0 + 128, :], ysb[:])

        if dbg:
            nc.sync.dma_start(qTn_d[:], qTn_sb[:])
            nc.sync.dma_start(kTn_d[:], kTn_sb[:])
            with tc.tile_pool(name="vdbg", bufs=1) as vdbg:
                vf = vdbg.tile([P, 16, HG * 65], f32)
                nc.vector.tensor_copy(vf[:], v_sb[:])
                nc.sync.dma_start(v_dd[:], vf[:])
            nc.sync.dma_start(oT_d[:], oT_sb[:])

    nc.compile()
    return nc


def _host_inputs(emb, context, Wq, Wk, Wv, Wu, qn_w, qn_b, kn_w, kn_b):
    bf16 = ml_dtypes.bfloat16
    redblk = np.zeros((P, 2), np.float32)
    redblk[0:64, 0] = 1.0 / 64.0
    redblk[64:128, 1] = 1.0 / 64.0
    redblk = redblk.astype(bf16)

    def center(Wrows):
        Wh = Wrows.reshape(HG, D, Wrows.shape[1])
        return (Wh - Wh.mean(axis=1, keepdims=True)).reshape(Wrows.shape)

    f32c = lambda a: np.ascontiguousarray(a, dtype=np.float32)
    tile2 = lambda w: np.ascontiguousarray(
        np.tile(np.asarray(w, np.float32), 2)[:, None])

    in_maps = []
    for c in range(8):
        b, hg = divmod(c, 4)
        rows = slice(INNER_C * hg, INNER_C * (hg + 1))
        in_maps.append({
            "embT": f32c(emb[b].T),
            "ctxT": f32c(context[b].T),
            "wqT": f32c(center(Wq[rows]).T),
            "wkT": f32c(center(Wk[rows]).T),
            "wvT": f32c(Wv[rows].T),
            "wuT": f32c(Wu[:, rows].T),
            "redblk": redblk,
            "qnw": tile2(qn_w),
            "qnb": tile2(qn_b),
            "knw": tile2(kn_w),
            "knb": tile2(kn_b),
        })
    return in_maps


def kernel(emb, context, Wq, Wk, Wv, Wu, bu, qn_w, qn_b, kn_w, kn_b):
    from concourse.bass_utils import run_bass_kernel_spmd

    global _cached_nc
    if _cached_nc is None:
        _cached_nc = _build()
    nc = _cached_nc

    emb = np.asarray(emb, np.float32)
    context = np.asarray(context, np.float32)
    in_maps = _host_inputs(np.asarray(emb), np.asarray(context),
                           np.asarray(Wq), np.asarray(Wk), np.asarray(Wv),
                           np.asarray(Wu), np.asarray(qn_w), np.asarray(qn_b),
                           np.asarray(kn_w), np.asarray(kn_b))

    trace = bool(os.environ.get("KERNEL_TRACE"))
    res = run_bass_kernel_spmd(nc, in_maps, core_ids=list(range(8)),
                               trace=trace)
    if trace:
        print(f"HW exec time: {res.exec_time_ns} ns")

    out = np.zeros((B, SQ, EMB), np.float32)
    for c in range(8):
        out[c // 4] += res.results[c]["ypart"]
    out += np.asarray(bu, np.float32)[None, None, :]
    return out


if __name__ == "__main__":
    rng = np.random.default_rng(0)
    pass
